# revision 3
# baseline (speedup 1.0000x reference)
"""MoE layer (T=16384, H=1024, F=4096, E=8, top-2) on 8 Trainium2 cores.

Expert parallelism (v5): core e owns expert e, processes C = max expert
count (rounded to 128) token slots. Weights are SBUF-resident in
compute-order layouts — w1 f-major and w2 h-major — so the first GEMM
starts as soon as one 256KB w1 tile and the first x chunk land (~8us)
instead of waiting for the full 16.8MB weight load. Outputs return bf16;
routing gates are applied on host in fp32.
"""

import numpy as np
import ml_dtypes

T, H, F, E, TOPK = 16384, 1024, 4096, 8, 2
P = 128
CHUNK = 512
KT = H // P                    # 8  k-tiles over H
FT = F // P                    # 32 tiles over F
HT = H // P                    # 8  output tiles over H

BF16 = ml_dtypes.bfloat16

_module_cache: dict = {}


def _capacity(max_cnt: int) -> int:
    return max(256, ((max_cnt + 127) // 128) * 128)


def _routing(x: np.ndarray, Wg: np.ndarray):
    """Top-2 expert ids and renormalized gates, matching the jax reference.

    The reference receives numpy arrays, so its `x @ Wg` runs through numpy
    BLAS — replicate that exactly (the expert ranking has 1-ulp knife-edge
    ties that flip between BLAS and XLA matmul). softmax/top_k then follow
    the reference's jax ops on CPU.
    """
    logits = x @ Wg  # numpy BLAS fp32, same as reference(**np_inputs)
    try:
        import jax
        import jax.numpy as jnp

        cpu = jax.devices("cpu")[0]
        with jax.default_device(cpu):
            lj = jax.device_put(jnp.asarray(logits), cpu)
            probs = jax.nn.softmax(lj, axis=-1)
            tv, ti = jax.lax.top_k(probs, TOPK)
            rw = tv / jnp.sum(tv, axis=-1, keepdims=True)
        return np.asarray(ti), np.asarray(rw, np.float32)
    except Exception:
        m = logits.max(axis=1, keepdims=True)
        p = np.exp(logits - m)
        p /= p.sum(axis=1, keepdims=True)
        order = np.argsort(-p, axis=1, kind="stable")
        ti = order[:, :TOPK]
        tv = np.take_along_axis(p, ti, axis=1)
        rw = (tv / tv.sum(axis=1, keepdims=True)).astype(np.float32)
        return ti, rw


def _build_module(C: int, repeat: int = 1):
    import concourse.mybir as mybir
    import concourse.tile as tile
    from concourse import bacc
    from concourse.bass import ts

    dt = mybir.dt
    assert C % 128 == 0
    chunk_sizes = [CHUNK] * (C // CHUNK)
    if C % CHUNK:
        chunk_sizes.append(C % CHUNK)
    chunk_offs = [sum(chunk_sizes[:i]) for i in range(len(chunk_sizes))]

    nc = bacc.Bacc("TRN2", target_bir_lowering=False, debug=False)

    xeT = nc.dram_tensor("xeT", (KT, P, C), dt.bfloat16, kind="ExternalInput").ap()
    w1 = nc.dram_tensor("w1", (FT, P, KT * P), dt.bfloat16, kind="ExternalInput").ap()
    w2 = nc.dram_tensor("w2", (HT, P, FT * P), dt.bfloat16, kind="ExternalInput").ap()
    yeT = nc.dram_tensor("yeT", (HT, P, C), dt.bfloat16, kind="ExternalOutput").ap()

    with tile.TileContext(nc) as tc:
        with (
            tc.tile_pool(name="wpool", bufs=1) as wpool,
            tc.tile_pool(name="xpool", bufs=2) as xpool,
            tc.tile_pool(name="hpool", bufs=1) as hpool,
            tc.tile_pool(name="opool", bufs=2) as opool,
            tc.tile_pool(name="spool", bufs=2) as spool,
            tc.tile_pool(name="ps1", bufs=4, space="PSUM") as ps1,
            tc.tile_pool(name="ps2", bufs=4, space="PSUM") as ps2,
        ):
            for rep in range(repeat):
                # startup: w1 f-tiles interleaved with x chunk 0, then w2
                w1r = wpool.tile([P, FT, KT * P], dt.bfloat16, tag="w1t")
                xt0 = xpool.tile([P, KT, CHUNK], dt.bfloat16, tag="xt")
                for f in range(FT):
                    nc.sync.dma_start(out=w1r[:, f, :], in_=w1[f, :, :])
                    if f < KT:
                        nc.sync.dma_start(
                            out=xt0[:, f, : chunk_sizes[0]],
                            in_=xeT[f, :, 0 : chunk_sizes[0]],
                        )
                w2r = wpool.tile([P, HT, FT * P], dt.bfloat16, tag="w2t")
                for h in range(HT):
                    nc.sync.dma_start(out=w2r[:, h, :], in_=w2[h, :, :])

                xts = xt0
                for j, (CH, off) in enumerate(zip(chunk_sizes, chunk_offs)):
                    xt_next = None
                    if j + 1 < len(chunk_sizes):
                        CH2, off2 = chunk_sizes[j + 1], chunk_offs[j + 1]
                        xt_next = xpool.tile([P, KT, CHUNK], dt.bfloat16, tag="xt")
                        for k in range(KT):
                            nc.sync.dma_start(
                                out=xt_next[:, k, :CH2],
                                in_=xeT[k, :, off2 : off2 + CH2],
                            )
                    ht = hpool.tile([P, FT, CHUNK], dt.bfloat16, tag="ht")
                    for f in range(FT):
                        ph = ps1.tile([P, CHUNK], dt.float32, tag="ph")
                        for k in range(KT):
                            nc.tensor.matmul(
                                ph[:, :CH],
                                lhsT=w1r[:, f, ts(k, P)],
                                rhs=xts[:, k, :CH],
                                start=(k == 0),
                                stop=(k == KT - 1),
                            )
                        # silu(x) = x * sigmoid(x); HW Silu LUT set is broken
                        # on this runtime (NRT_EXEC_UNIT_UNRECOVERABLE).
                        sg = spool.tile([P, CHUNK], dt.float32, tag="sg")
                        nc.scalar.activation(
                            sg[:, :CH], ph[:, :CH],
                            mybir.ActivationFunctionType.Sigmoid,
                        )
                        nc.vector.tensor_mul(ht[:, f, :CH], sg[:, :CH], ph[:, :CH])
                    for h in range(HT):
                        py = ps2.tile([P, CHUNK], dt.float32, tag="py")
                        for f in range(FT):
                            nc.tensor.matmul(
                                py[:, :CH],
                                lhsT=w2r[:, h, ts(f, P)],
                                rhs=ht[:, f, :CH],
                                start=(f == 0),
                                stop=(f == FT - 1),
                            )
                        ot = opool.tile([P, CHUNK], dt.bfloat16, tag="ot")
                        nc.vector.tensor_copy(ot[:, :CH], py[:, :CH])
                        nc.sync.dma_start(
                            out=yeT[h, :, off : off + CH], in_=ot[:, :CH]
                        )
                    xts = xt_next

    nc.compile()
    return nc


def _get_module(C: int, repeat: int = 1, full_reload: bool = True):
    key = (C, repeat)
    if key not in _module_cache:
        _module_cache[key] = _build_module(C, repeat)
    return _module_cache[key]


def _fmajor_w1(w1e: np.ndarray) -> np.ndarray:
    """[H, F] -> (FT, P, KT*P) with [f, p, k*P+c] = w1e[k*P+p, f*P+c]."""
    return np.ascontiguousarray(
        w1e.astype(BF16).reshape(KT, P, FT, P).transpose(2, 1, 0, 3)
    ).reshape(FT, P, KT * P)


def _hmajor_w2(w2e: np.ndarray) -> np.ndarray:
    """[F, H] -> (HT, P, FT*P) with [h, p, f*P+c] = w2e[f*P+p, h*P+c]."""
    return np.ascontiguousarray(
        w2e.astype(BF16).reshape(FT, P, HT, P).transpose(2, 1, 0, 3)
    ).reshape(HT, P, FT * P)


def _plan(ti: np.ndarray, rw: np.ndarray):
    idx_list, gate_list = [], []
    for e in range(E):
        hit = ti == e
        rows = np.nonzero(hit.any(axis=1))[0]
        g = np.where(hit[rows, 0], rw[rows, 0], rw[rows, 1]).astype(np.float32)
        idx_list.append(rows)
        gate_list.append(g)
    return idx_list, gate_list


def _in_maps(x: np.ndarray, w1: np.ndarray, w2: np.ndarray, idx_list, C: int):
    in_maps = []
    for e in range(E):
        rows = idx_list[e]
        xeT = np.zeros((H, C), BF16)
        xeT[:, : len(rows)] = x[rows].T.astype(BF16)
        in_maps.append(
            {
                "xeT": xeT.reshape(KT, P, C),
                "w1": _fmajor_w1(w1[e]),
                "w2": _hmajor_w2(w2[e]),
            }
        )
    return in_maps


def kernel(x: np.ndarray, Wg: np.ndarray, w1: np.ndarray, w2: np.ndarray,
           **_unused) -> np.ndarray:
    from concourse.bass_utils import run_bass_kernel_spmd

    x = np.ascontiguousarray(np.asarray(x, np.float32))
    Wg = np.asarray(Wg, np.float32)
    w1 = np.asarray(w1, np.float32)
    w2 = np.asarray(w2, np.float32)
    nt = x.shape[0]

    ti, rw = _routing(x, Wg)
    idx_list, gate_list = _plan(ti, rw)
    C = _capacity(max(len(r) for r in idx_list))
    nc = _get_module(C)
    in_maps = _in_maps(x, w1, w2, idx_list, C)

    res = run_bass_kernel_spmd(nc, in_maps, core_ids=list(range(E)))

    y = np.zeros((nt, H), np.float32)
    for e in range(E):
        rows = idx_list[e]
        ye = res.results[e]["yeT"].reshape(H, C)[:, : len(rows)]
        y[rows] += gate_list[e][:, None] * ye.T.astype(np.float32)
    return y


if __name__ == "__main__":
    rng = np.random.default_rng(0)
    xs = rng.standard_normal((T, H), dtype=np.float32)
    Wgs = rng.standard_normal((H, E), dtype=np.float32) / np.sqrt(H)
    w1s = rng.standard_normal((E, H, F), dtype=np.float32) / np.sqrt(H)
    w2s = rng.standard_normal((E, F, H), dtype=np.float32) / np.sqrt(F)
    out = kernel(x=xs, Wg=Wgs, w1=w1s, w2=w2s)
    print(out.shape, out.dtype)


# revision 4
# speedup vs baseline: 1.5178x; 1.5178x over previous
"""MoE layer (T=16384, H=1024, F=4096, E=8, top-2) on 8 Trainium2 cores.

Expert parallelism with load-balanced overflow (v6):
  - Core e owns expert e and processes exactly C_RES=4096 resident token
    slots (the per-core mean) instead of the max expert count. Overflow
    tokens of over-subscribed experts are packed into 128-token blocks
    and spread across cores with the owning expert's weights streamed
    from DRAM — the all-to-all of expert parallelism, done on the host.
  - Resident w1/w2 stay SBUF-resident in compute-order layouts (w1
    f-major, w2 h-major) so the first GEMM starts ~8us in.
  - Overflow GEMM1 runs at chunk boundaries 0-3 (8 f-steps each) with
    2KB weight tiles prefetched one full chunk ahead; overflow GEMM2
    runs 2 h-steps per boundary at boundaries 4-6 plus 2 at the tail,
    with 2KB quarter-tiles of the foreign w2 streamed the same way.
    Ring DMAs are issued only where their slot is already free — a
    WAR-blocked DMA at the head of Trn2's single HWDGE FIFO blocks all
    later DMAs (this is what sank the streamed-w2 variants).
  - bf16 matmuls accumulate in fp32 PSUM; outputs return bf16; routing
    gates are applied on host in fp32.
"""

import numpy as np
import ml_dtypes

T, H, F, E, TOPK = 16384, 1024, 4096, 8, 2
P = 128
C_RES = T * TOPK // E          # 4096 resident token slots per core
BLK = 96                       # overflow block size (tokens)
CHUNK = 512
KT = H // P                    # 8  k-tiles over H
FT = F // P                    # 32 tiles over F
HT = H // P                    # 8  output tiles over H
QF = 8                         # f-tiles per streamed w2 quarter
NQ = FT // QF                  # 4  quarters per w2 h-tile
RING = 8                       # streamed-tile ring slots

BF16 = ml_dtypes.bfloat16

_module_cache: dict = {}


def _routing(x: np.ndarray, Wg: np.ndarray):
    """Top-2 expert ids and renormalized gates, matching the jax reference.

    The reference receives numpy arrays, so its `x @ Wg` runs through numpy
    BLAS — replicate that exactly (the expert ranking has 1-ulp knife-edge
    ties that flip between BLAS and XLA matmul). softmax/top_k then follow
    the reference's jax ops on CPU.
    """
    logits = x @ Wg  # numpy BLAS fp32, same as reference(**np_inputs)
    try:
        import jax
        import jax.numpy as jnp

        cpu = jax.devices("cpu")[0]
        with jax.default_device(cpu):
            lj = jax.device_put(jnp.asarray(logits), cpu)
            probs = jax.nn.softmax(lj, axis=-1)
            tv, ti = jax.lax.top_k(probs, TOPK)
            rw = tv / jnp.sum(tv, axis=-1, keepdims=True)
        return np.asarray(ti), np.asarray(rw, np.float32)
    except Exception:
        m = logits.max(axis=1, keepdims=True)
        p = np.exp(logits - m)
        p /= p.sum(axis=1, keepdims=True)
        order = np.argsort(-p, axis=1, kind="stable")
        ti = order[:, :TOPK]
        tv = np.take_along_axis(p, ti, axis=1)
        rw = (tv / tv.sum(axis=1, keepdims=True)).astype(np.float32)
        return ti, rw


def _schedule_overflow(S: int, nb: int):
    """Boundary work lists. Each boundary holds at most RING streamed
    tiles: either one batch of G1 f-steps or G2 h-steps (NQ quarters
    each). Whatever doesn't fit runs at the tail."""
    g1 = [(s, f) for s in range(S) for f in range(FT)]
    g2 = [(s, h) for s in range(S) for h in range(HT)]
    parts = []  # per boundary: ("g1"|"g2", [steps])
    i1 = 0
    while i1 < len(g1) and len(parts) < nb:
        parts.append(("g1", g1[i1 : i1 + RING]))
        i1 += RING
    tail_g1 = g1[i1:]
    i2 = 0
    if not tail_g1:
        while i2 < len(g2) and len(parts) < nb:
            parts.append(("g2", g2[i2 : i2 + RING // NQ]))
            i2 += RING // NQ
    tail_g2 = g2[i2:]
    parts += [("g1", [])] * (nb - len(parts))
    return parts, tail_g1, tail_g2


def _build_module(S: int, repeat: int = 1, full_reload: bool = True):
    import concourse.mybir as mybir
    import concourse.tile as tile
    from concourse import bacc
    from concourse.bass import ts

    dt = mybir.dt
    assert C_RES % CHUNK == 0
    NCH = C_RES // CHUNK
    SW = max(S, 1)

    nc = bacc.Bacc("TRN2", target_bir_lowering=False, debug=False)

    xeT = nc.dram_tensor("xeT", (KT, P, C_RES), dt.bfloat16, kind="ExternalInput").ap()
    w1 = nc.dram_tensor("w1", (FT, P, KT * P), dt.bfloat16, kind="ExternalInput").ap()
    w2 = nc.dram_tensor("w2", (HT, P, FT * P), dt.bfloat16, kind="ExternalInput").ap()
    xsT = nc.dram_tensor("xsT", (KT, P, SW * BLK), dt.bfloat16, kind="ExternalInput").ap()
    ws1 = nc.dram_tensor("ws1", (SW * FT, P, KT * P), dt.bfloat16, kind="ExternalInput").ap()
    ws2 = nc.dram_tensor("ws2", (SW * HT, P, FT * P), dt.bfloat16, kind="ExternalInput").ap()
    yeT = nc.dram_tensor("yeT", (HT, P, C_RES), dt.bfloat16, kind="ExternalOutput").ap()
    ysT = nc.dram_tensor("ysT", (HT, P, SW * BLK), dt.bfloat16, kind="ExternalOutput").ap()

    parts, tail_g1, tail_g2 = _schedule_overflow(S, NCH - 1)

    with tile.TileContext(nc) as tc:
        with (
            tc.tile_pool(name="wpool", bufs=1) as wpool,
            tc.tile_pool(name="xpool", bufs=2) as xpool,
            tc.tile_pool(name="hpool", bufs=1) as hpool,
            tc.tile_pool(name="hspool", bufs=1) as hspool,
            tc.tile_pool(name="xspool", bufs=1) as xspool,
            tc.tile_pool(name="ring", bufs=RING) as ring,
            tc.tile_pool(name="spool", bufs=2) as spool,
            tc.tile_pool(name="opool", bufs=2) as opool,
            tc.tile_pool(name="ps1", bufs=4, space="PSUM") as ps1,
            tc.tile_pool(name="ps2", bufs=4, space="PSUM") as ps2,
        ):
            def issue_part(part, tiles):
                """DMA one boundary's streamed tiles (slots are free here)."""
                kind, steps = part
                if kind == "g1":
                    for (s, f) in steps:
                        wt = ring.tile([P, KT * P], dt.bfloat16,
                                       name="rt", tag="rt")
                        nc.sync.dma_start(out=wt[:], in_=ws1[s * FT + f, :, :])
                        tiles[("g1", s, f)] = wt
                else:
                    for (s, h) in steps:
                        for q in range(NQ):
                            wt = ring.tile([P, QF * P], dt.bfloat16,
                                           name="rt", tag="rt")
                            nc.sync.dma_start(
                                out=wt[:],
                                in_=ws2[s * HT + h, :, q * QF * P : (q + 1) * QF * P],
                            )
                            tiles[("g2", s, h, q)] = wt

            def g1_step(s, f, wt, xs, hs):
                ph = ps1.tile([P, BLK], dt.float32, tag="phb", bufs=2)
                for k in range(KT):
                    nc.tensor.matmul(
                        ph[:, :BLK],
                        lhsT=wt[:, ts(k, P)],
                        rhs=xs[:, k, s * BLK : (s + 1) * BLK],
                        start=(k == 0),
                        stop=(k == KT - 1),
                    )
                sg = spool.tile([P, BLK], dt.float32, tag="sgb")
                nc.scalar.activation(
                    sg[:, :BLK], ph[:, :BLK],
                    mybir.ActivationFunctionType.Sigmoid,
                )
                nc.vector.tensor_mul(
                    hs[:, f, s * BLK : (s + 1) * BLK], sg[:, :BLK], ph[:, :BLK]
                )

            def g2_step(s, h, quarters, hs):
                py = ps1.tile([P, BLK], dt.float32, tag="phb", bufs=2)
                for q in range(NQ):
                    for fl in range(QF):
                        f = q * QF + fl
                        nc.tensor.matmul(
                            py[:, :BLK],
                            lhsT=quarters[q][:, ts(fl, P)],
                            rhs=hs[:, f, s * BLK : (s + 1) * BLK],
                            start=(f == 0),
                            stop=(f == FT - 1),
                        )
                ot = opool.tile([P, CHUNK], dt.bfloat16, tag="ot")
                nc.vector.tensor_copy(ot[:, :BLK], py[:, :BLK])
                nc.sync.dma_start(
                    out=ysT[h, :, s * BLK : (s + 1) * BLK], in_=ot[:, :BLK]
                )

            def run_part(part, tiles, xs, hs):
                kind, steps = part
                if kind == "g1":
                    for (s, f) in steps:
                        g1_step(s, f, tiles.pop(("g1", s, f)), xs, hs)
                else:
                    for (s, h) in steps:
                        quarters = [tiles.pop(("g2", s, h, q)) for q in range(NQ)]
                        g2_step(s, h, quarters, hs)

            for rep in range(repeat):
                # startup: w1 f-tiles interleaved with x chunk 0, then w2
                w1r = wpool.tile([P, FT, KT * P], dt.bfloat16, tag="w1t")
                xt0 = xpool.tile([P, KT, CHUNK], dt.bfloat16, tag="xt")
                for f in range(FT):
                    nc.sync.dma_start(out=w1r[:, f, :], in_=w1[f, :, :])
                    if f < KT:
                        nc.sync.dma_start(out=xt0[:, f, :], in_=xeT[f, :, 0:CHUNK])
                w2r = wpool.tile([P, HT, FT * P], dt.bfloat16, tag="w2t")
                for h in range(HT):
                    nc.sync.dma_start(out=w2r[:, h, :], in_=w2[h, :, :])

                xs = hs = None
                if S > 0:
                    xs = xspool.tile([P, KT, SW * BLK], dt.bfloat16, tag="xs")
                    for k in range(KT):
                        nc.sync.dma_start(out=xs[:, k, :], in_=xsT[k, :, :])
                    hs = hspool.tile([P, FT, SW * BLK], dt.bfloat16, tag="hs")

                tiles = {}
                if S > 0 and parts:
                    issue_part(parts[0], tiles)  # boundary-0 tiles

                xts = xt0
                for j in range(NCH):
                    off = j * CHUNK
                    xt_next = None
                    if j + 1 < NCH:
                        xt_next = xpool.tile([P, KT, CHUNK], dt.bfloat16, tag="xt")
                        for k in range(KT):
                            nc.sync.dma_start(
                                out=xt_next[:, k, :],
                                in_=xeT[k, :, off + CHUNK : off + 2 * CHUNK],
                            )
                    # ---- GEMM1 ----
                    ht = hpool.tile([P, FT, CHUNK], dt.bfloat16, tag="ht")
                    for f in range(FT):
                        ph = ps1.tile([P, CHUNK], dt.float32, tag="ph", bufs=3)
                        for k in range(KT):
                            nc.tensor.matmul(
                                ph[:],
                                lhsT=w1r[:, f, ts(k, P)],
                                rhs=xts[:, k, :],
                                start=(k == 0),
                                stop=(k == KT - 1),
                            )
                        # silu(x) = x * sigmoid(x); the HW Silu LUT set is
                        # broken on this runtime (NRT_EXEC_UNIT_UNRECOVERABLE).
                        sg = spool.tile([P, CHUNK], dt.float32, tag="sg")
                        nc.scalar.activation(
                            sg[:], ph[:], mybir.ActivationFunctionType.Sigmoid
                        )
                        nc.vector.tensor_mul(ht[:, f, :], sg[:], ph[:])
                    # ---- GEMM2 ----
                    for h in range(HT):
                        py = ps2.tile([P, CHUNK], dt.float32, tag="py", bufs=3)
                        for f in range(FT):
                            nc.tensor.matmul(
                                py[:],
                                lhsT=w2r[:, h, ts(f, P)],
                                rhs=ht[:, f, :],
                                start=(f == 0),
                                stop=(f == FT - 1),
                            )
                        ot = opool.tile([P, CHUNK], dt.bfloat16, tag="ot")
                        nc.vector.tensor_copy(ot[:], py[:])
                        nc.sync.dma_start(
                            out=yeT[h, :, off : off + CHUNK], in_=ot[:]
                        )
                    # ---- boundary: overflow steps, then next batch's DMAs ----
                    if S > 0 and j < len(parts):
                        run_part(parts[j], tiles, xs, hs)
                        if j + 1 < len(parts):
                            issue_part(parts[j + 1], tiles)
                        elif j + 1 == len(parts):
                            # prefetch tail quarters into the freed ring
                            for i, (s, h) in enumerate(tail_g2[: RING // NQ]):
                                issue_part(("g2", [(s, h)]), tiles)
                    xts = xt_next

                # ---- tail: leftover overflow ----
                for (s, f) in tail_g1:
                    wt = ring.tile([P, KT * P], dt.bfloat16, name="rt", tag="rt")
                    nc.sync.dma_start(out=wt[:], in_=ws1[s * FT + f, :, :])
                    g1_step(s, f, wt, xs, hs)
                for i, (s, h) in enumerate(tail_g2):
                    if ("g2", s, h, 0) not in tiles:
                        issue_part(("g2", [(s, h)]), tiles)
                    quarters = [tiles.pop(("g2", s, h, q)) for q in range(NQ)]
                    g2_step(s, h, quarters, hs)

    nc.compile()
    return nc


def _get_module(S: int, repeat: int = 1, full_reload: bool = True):
    key = (S, repeat)
    if key not in _module_cache:
        _module_cache[key] = _build_module(S, repeat)
    return _module_cache[key]


def _plan(ti: np.ndarray, rw: np.ndarray):
    """Dispatch plan: resident rows/gates per expert + overflow blocks."""
    idx_list, gate_list = [], []
    for e in range(E):
        hit = ti == e
        rows = np.nonzero(hit.any(axis=1))[0]
        g = np.where(hit[rows, 0], rw[rows, 0], rw[rows, 1]).astype(np.float32)
        idx_list.append(rows)
        gate_list.append(g)

    blocks = []
    for e in range(E):
        n = len(idx_list[e])
        for off in range(C_RES, n, BLK):
            blocks.append(
                (e, idx_list[e][off : off + BLK], gate_list[e][off : off + BLK])
            )
    S = (len(blocks) + E - 1) // E
    return idx_list, gate_list, blocks, S


def _fmajor_w1(w1e: np.ndarray) -> np.ndarray:
    """[H, F] -> (FT, P, KT*P) with [f, p, k*P+c] = w1e[k*P+p, f*P+c]."""
    return np.ascontiguousarray(
        w1e.astype(BF16).reshape(KT, P, FT, P).transpose(2, 1, 0, 3)
    ).reshape(FT, P, KT * P)


def _hmajor_w2(w2e: np.ndarray) -> np.ndarray:
    """[F, H] -> (HT, P, FT*P) with [h, p, f*P+c] = w2e[f*P+p, h*P+c]."""
    return np.ascontiguousarray(
        w2e.astype(BF16).reshape(FT, P, HT, P).transpose(2, 1, 0, 3)
    ).reshape(HT, P, FT * P)


def _in_maps(x: np.ndarray, w1: np.ndarray, w2: np.ndarray,
             idx_list, blocks, S: int):
    SW = max(S, 1)
    w1b = [None] * E
    w2b = [None] * E

    def get_w(e):
        if w1b[e] is None:
            w1b[e] = _fmajor_w1(w1[e])
            w2b[e] = _hmajor_w2(w2[e])
        return w1b[e], w2b[e]

    in_maps = []
    for c in range(E):
        rows = idx_list[c][:C_RES]
        xeT = np.zeros((H, C_RES), BF16)
        xeT[:, : len(rows)] = x[rows].T.astype(BF16)
        w1c, w2c = get_w(c)
        xsT = np.zeros((H, SW * BLK), BF16)
        ws1 = np.zeros((SW * FT, P, KT * P), BF16)
        ws2 = np.zeros((SW * HT, P, FT * P), BF16)
        for s in range(S):
            b = c * S + s
            if b < len(blocks):
                e, brows, _ = blocks[b]
                xsT[:, s * BLK : s * BLK + len(brows)] = x[brows].T.astype(BF16)
                w1s, w2s = get_w(e)
                ws1[s * FT : (s + 1) * FT] = w1s
                ws2[s * HT : (s + 1) * HT] = w2s
        in_maps.append(
            {
                "xeT": xeT.reshape(KT, P, C_RES),
                "w1": w1c,
                "w2": w2c,
                "xsT": xsT.reshape(KT, P, SW * BLK),
                "ws1": ws1,
                "ws2": ws2,
            }
        )
    return in_maps


def kernel(x: np.ndarray, Wg: np.ndarray, w1: np.ndarray, w2: np.ndarray,
           **_unused) -> np.ndarray:
    from concourse.bass_utils import run_bass_kernel_spmd

    x = np.ascontiguousarray(np.asarray(x, np.float32))
    Wg = np.asarray(Wg, np.float32)
    w1 = np.asarray(w1, np.float32)
    w2 = np.asarray(w2, np.float32)
    nt = x.shape[0]

    ti, rw = _routing(x, Wg)
    idx_list, gate_list, blocks, S = _plan(ti, rw)
    nc = _get_module(S)
    in_maps = _in_maps(x, w1, w2, idx_list, blocks, S)

    res = run_bass_kernel_spmd(nc, in_maps, core_ids=list(range(E)))

    y = np.zeros((nt, H), np.float32)
    for c in range(E):
        rows = idx_list[c][:C_RES]
        ye = res.results[c]["yeT"].reshape(H, C_RES)[:, : len(rows)]
        y[rows] += gate_list[c][: len(rows), None] * ye.T.astype(np.float32)
        ys = res.results[c]["ysT"].reshape(H, max(S, 1) * BLK)
        for s in range(S):
            b = c * S + s
            if b < len(blocks):
                _, brows, bg = blocks[b]
                yb = ys[:, s * BLK : s * BLK + len(brows)]
                y[brows] += bg[:, None] * yb.T.astype(np.float32)
    return y


if __name__ == "__main__":
    rng = np.random.default_rng(0)
    xs = rng.standard_normal((T, H), dtype=np.float32)
    Wgs = rng.standard_normal((H, E), dtype=np.float32) / np.sqrt(H)
    w1s = rng.standard_normal((E, H, F), dtype=np.float32) / np.sqrt(H)
    w2s = rng.standard_normal((E, F, H), dtype=np.float32) / np.sqrt(F)
    out = kernel(x=xs, Wg=Wgs, w1=w1s, w2=w2s)
    print(out.shape, out.dtype)
